# revision 5
# baseline (speedup 1.0000x reference)
"""Trainium2 Bass kernel for one burst-mode CIF neuron step.

Reference math (closed form of the two burst while-loops):
    m      = mem + x
    q      = m / th
    k_pos  = max(ceil(q) - 1, 0)
    j_mem  = max(-floor(q) - 1, 0)          (mutually exclusive with k_pos)
    k_neg  = min(j_mem, round(spike_count/th))
    spike  = (k_pos - k_neg) * th

Device reformulation.  With g = ceil(q) (computed via the fp32
round-to-nearest magic constant C = 1.5*2^23: ts2 = (q + 0.5) + C
= C + ceil(q) a.e.), s = spike_count/th >= 0:
    kp      = relu(g - 1)      = relu(ts2 - (C+1))
    kn_neg  = -k_neg = max(min(g, 0), -s)
    spike   = th * (kp + kn_neg)

Layout: TRANSPOSED so the hidden dim lives on partitions.  Rows
(B*T = 16384) are sharded 8-way data-parallel (2048 rows/core = the
free dim); H = 4096 becomes 32 partition-blocks of 128.  Threshold is
then a per-partition [128,1] scalar per block, so every (1/th) multiply
fuses into tensor_scalar / ACT-scale operands instead of needing
broadcast [128,H] tensor_tensor ops (which dominated the previous
row-major kernel).

Dtypes: x/mem are sent as f16 (halves input traffic; measured end-to-end
L2 rel err 9.5e-3 vs the f32 reference, dominated by ceil-boundary flips
from the f16 quantization - gate is 2e-2).  spike_count is sent bf16
(counts <= 3 exactly recoverable).  Output is bf16, upcast on host.

Per [128, 2048] block (one per h-chunk), engine split:
    DVE : m = x+mem (f16 2x), jn = min(ts2-C, 0), sn = -R*sc,
          kn_neg = max(jn, sn)
    ACT : ta = Ident(R*m + 0.5), out = Ident(th * psum) PSUM->SBUF
    GP  : ts2 = ta + C, kp = max(ts2-(C+1), 0)
    PE  : psum = I.T@kp + I.T@kn_neg  (exact: small ints in bf16)
Input arrives packed [x | mem | sc] per partition-row so each block is
ONE contiguous 1.5MB DMA; output is one 512KB DMA per block.
"""

import numpy as np

B, T, H = 4, 4096, 4096
N_CORES = 8
R_TOTAL = B * T            # 16384 rows
FD = R_TOTAL // N_CORES    # 2048 rows per core = free dim
P = 128
NB = H // P                # 32 h-blocks per core
MAGIC = 12582912.0         # C = 1.5 * 2^23
NMM = 512                  # matmul free-dim per PSUM bank

_NC_CACHE: dict = {}


def build_nc():
    """Build the per-core Bass program (identical on all cores)."""
    from contextlib import ExitStack

    import concourse.bacc as bacc
    import concourse.bass as bass
    import concourse.mybir as mybir
    from concourse.tile import TileContext

    f32 = mybir.dt.float32
    f16 = mybir.dt.float16
    bf16 = mybir.dt.bfloat16
    Alu = mybir.AluOpType
    Act = mybir.ActivationFunctionType

    nc = bacc.Bacc("TRN2", target_bir_lowering=False, debug=False)
    xm_d = nc.dram_tensor("xm", [NB * P, 3 * FD], bf16, kind="ExternalInput").ap()
    th_d = nc.dram_tensor("thp", [P, NB], f32, kind="ExternalInput").ap()
    e_d = nc.dram_tensor("eye", [P, P], bf16, kind="ExternalInput").ap()
    o_d = nc.dram_tensor("spike", [NB * P, FD], bf16, kind="ExternalOutput").ap()

    with TileContext(nc) as tc, ExitStack() as ctx:
        consts = ctx.enter_context(tc.tile_pool(name="consts", bufs=1))
        io = ctx.enter_context(tc.tile_pool(name="io", bufs=4))
        pm = ctx.enter_context(tc.tile_pool(name="pm", bufs=2))
        pa = ctx.enter_context(tc.tile_pool(name="pa", bufs=2))
        pt = ctx.enter_context(tc.tile_pool(name="pt", bufs=2))
        pk = ctx.enter_context(tc.tile_pool(name="pk", bufs=2))
        pj = ctx.enter_context(tc.tile_pool(name="pj", bufs=2))
        ps = ctx.enter_context(tc.tile_pool(name="ps", bufs=2))
        po = ctx.enter_context(tc.tile_pool(name="po", bufs=3))
        psum = ctx.enter_context(tc.tile_pool(name="psum", bufs=2, space="PSUM"))

        # ---- one-time setup ----
        TH = consts.tile([P, NB], f32, tag="TH")
        nc.sync.dma_start(out=TH[:], in_=th_d)
        Rr = consts.tile([P, NB], f32, tag="Rr")
        nc.vector.reciprocal(Rr[:], TH[:])
        Rn = consts.tile([P, NB], f32, tag="Rn")
        nc.vector.tensor_scalar_mul(Rn[:], Rr[:], -1.0)
        eye = consts.tile([P, P], bf16, tag="eye")
        nc.sync.dma_start(out=eye[:], in_=e_d)
        bias_half = consts.tile([P, 1], f32, tag="bias_half")
        nc.vector.memset(bias_half[:], 0.5)
        bias_zero = consts.tile([P, 1], f32, tag="bias_zero")
        nc.vector.memset(bias_zero[:], 0.0)

        xm_t = xm_d.rearrange("(nb p) w -> nb p w", p=P)
        o_t = o_d.rearrange("(nb p) w -> nb p w", p=P)

        for b in range(NB):
            txm = io.tile([P, 3 * FD], bf16, tag="xm")
            nc.sync.dma_start(out=txm[:], in_=xm_t[b])
            xpart = txm[:, 0:FD].bitcast(f16)
            mpart = txm[:, FD : 2 * FD].bitcast(f16)
            scp = txm[:, 2 * FD : 3 * FD]

            # m = x + mem  (DVE, f16 2x mode)
            tm = pm.tile([P, FD], f16, tag="m")
            nc.vector.tensor_tensor(tm[:], xpart, mpart, Alu.add)
            # ta = q + 0.5 = R*m + 0.5  (ACT, per-partition scale)
            ta = pa.tile([P, FD], f32, tag="ta")
            nc.scalar.activation(
                ta[:], tm[:], Act.Identity, bias=bias_half[:], scale=Rr[:, b : b + 1]
            )
            # ts2 = ta + C = C + ceil(q)  (GPSIMD)
            ts2 = pt.tile([P, FD], f32, tag="ts2")
            nc.gpsimd.tensor_scalar(ts2[:], ta[:], MAGIC, None, Alu.add)
            # kp = relu(ts2 - (C+1))  (GPSIMD two-op tensor_scalar)
            kp = pk.tile([P, FD], bf16, tag="kp")
            nc.gpsimd.tensor_scalar(
                kp[:], ts2[:], MAGIC + 1.0, 0.0, Alu.subtract, Alu.max
            )
            # jn = min(ts2 - C, 0) = min(g, 0)  (DVE)
            jn = pj.tile([P, FD], bf16, tag="jn")
            nc.vector.tensor_scalar(jn[:], ts2[:], MAGIC, 0.0, Alu.subtract, Alu.min)
            # sn = -s = sc * (-R)  (DVE, bf16 4x)
            sn = ps.tile([P, FD], bf16, tag="sn")
            nc.vector.tensor_scalar(sn[:], scp, Rn[:, b : b + 1], None, Alu.mult)
            # kn_neg = max(jn, sn)  (DVE, in place)
            nc.vector.tensor_tensor(jn[:], jn[:], sn[:], Alu.max)
            # psum = I.T@kp + I.T@kn_neg = kp + kn_neg  (PE; exact small ints)
            td = psum.tile([P, FD], f32, tag="td")
            for c in range(FD // NMM):
                cs = bass.ts(c, NMM)
                nc.tensor.matmul(td[:, cs], eye[:], kp[:, cs], start=True, stop=False)
                nc.tensor.matmul(td[:, cs], eye[:], jn[:, cs], start=False, stop=True)
            # spike = th * d  (ACT: PSUM->SBUF copy with per-partition scale)
            tout = po.tile([P, FD], bf16, tag="out")
            nc.scalar.activation(
                tout[:], td[:], Act.Identity, bias=bias_zero[:], scale=TH[:, b : b + 1]
            )
            nc.scalar.dma_start(out=o_t[b], in_=tout[:])

    return nc


def make_in_maps(x, mem, sc, th):
    """Pack full [R_TOTAL, H] inputs into per-core transposed tensors.

    Per core: xm[b, p, :] = [x_f16 | mem_f16 | sc_bf16] for hidden channel
    h = b*128+p over that core's 2048 rows, so each block is one
    contiguous DMA.
    """
    import ml_dtypes

    x16 = x.astype(np.float16)
    m16 = mem.astype(np.float16)
    s16 = sc.astype(ml_dtypes.bfloat16)
    thp = np.ascontiguousarray(th.reshape(NB, P).T)  # [P, NB] f32
    eye = np.eye(P, dtype=np.float32).astype(ml_dtypes.bfloat16)

    in_maps = []
    for c in range(N_CORES):
        rs = slice(c * FD, (c + 1) * FD)
        pkd = np.empty((NB, P, 3 * FD), dtype=np.uint16)
        pkd[:, :, 0:FD] = x16[rs].view(np.uint16).reshape(FD, NB, P).transpose(1, 2, 0)
        pkd[:, :, FD : 2 * FD] = (
            m16[rs].view(np.uint16).reshape(FD, NB, P).transpose(1, 2, 0)
        )
        pkd[:, :, 2 * FD : 3 * FD] = (
            s16[rs].view(np.uint16).reshape(FD, NB, P).transpose(1, 2, 0)
        )
        in_maps.append(
            {
                "xm": pkd.reshape(NB * P, 3 * FD).view(ml_dtypes.bfloat16),
                "thp": thp,
                "eye": eye,
            }
        )
    return in_maps


def unpack_out(results):
    """results[c]["spike"] [NB*P, FD] bf16 -> full [B, T, H] f32."""
    outs = []
    for c in range(N_CORES):
        sp = np.asarray(results[c]["spike"]).astype(np.float32)
        # [NB, P, FD] -> [FD, NB, P] -> [FD, H]
        outs.append(sp.reshape(NB, P, FD).transpose(2, 0, 1).reshape(FD, H))
    return np.concatenate(outs, axis=0).reshape(B, T, H)


def kernel(**inputs: np.ndarray) -> np.ndarray:
    from concourse.bass_utils import run_bass_kernel_spmd

    x = np.ascontiguousarray(inputs["x"], dtype=np.float32).reshape(R_TOTAL, H)
    mem = np.ascontiguousarray(inputs["mem"], dtype=np.float32).reshape(R_TOTAL, H)
    sc = np.ascontiguousarray(inputs["spike_count"], dtype=np.float32).reshape(
        R_TOTAL, H
    )
    th = np.ascontiguousarray(inputs["threshold"], dtype=np.float32)

    if "nc" not in _NC_CACHE:
        nc = build_nc()
        nc.finalize()
        _NC_CACHE["nc"] = nc
    nc = _NC_CACHE["nc"]

    in_maps = make_in_maps(x, mem, sc, th)
    res = run_bass_kernel_spmd(nc, in_maps, core_ids=list(range(N_CORES)))
    return unpack_out(res.results)


# revision 6
# speedup vs baseline: 12.5542x; 12.5542x over previous
"""Trainium2 Bass kernel for one burst-mode CIF neuron step.

Reference math (closed form of the two burst while-loops):
    m      = mem + x
    q      = m / th
    k_pos  = max(ceil(q) - 1, 0)
    j_mem  = max(-floor(q) - 1, 0)          (mutually exclusive with k_pos)
    k_neg  = min(j_mem, round(spike_count/th))
    spike  = (k_pos - k_neg) * th

Device reformulation.  Let g = ceil(q) = rint(q + 0.5) a.e. and
s = spike_count/th >= 0.  Then
    k_pos   = relu(g - 1)
    -k_neg  = max(min(g, 0), -s)
    spike   = th * (k_pos - k_neg)

The rint rides the f16 OUTPUT ROUNDING of one ACT op: for |v| < 512,
f16(v + 1536) = 1536 + rint(v) (f16 ulp is 1.0 on [1024, 2048)).  So
    ta_b = f16(R*m + 1536.5)        = 1536 + g          (one ACT op)
    kp_b = max(ta_b, 1537)          = 1537 + k_pos      (DVE TS, 4x)
    jn_b = min(ta_b, 1536)          = 1536 + min(g,0)   (DVE TS, 4x)
    sn_b = f16(sc*(-R) + 1536)      = 1536 - s          (DVE TS, 4x)
    kn_b = max(jn_b, sn_b)          = 1536 - k_neg      (DVE TT, 2x)
    psum = I.T@kp_b + I.T@kn_b      = 3073 + d          (PE, exact ints)
    out  = bf16(th*psum - 3073*th)  = th * d            (ACT, per-part bias)
All intermediates are exact small integers (+bias) in f16; the only
rounding error sources are the f16 input quantization and the bf16
output (measured end-to-end rel err 1.1e-2 vs the f32 reference,
gate 2e-2).

Layout: TRANSPOSED so the hidden dim lives on partitions.  Rows
(B*T = 16384) are sharded 8-way data-parallel (2048 rows/core = free
dim); H = 4096 becomes 32 partition-blocks of 128.  Threshold is then a
per-partition [128,1] scalar per block, so every (1/th) multiply fuses
into tensor_scalar / ACT scale-bias operands.  Input arrives packed
[x | mem | sc] per partition-row: one contiguous 1.5MB DMA per block.

GPSIMD is deliberately IDLE: its tensor_scalar ucode runs ~21 cyc/elem
(~36us per block) and, while active, starves the DVE via the shared
SBUF port (measured: identical DVE ops 1.45us -> 35us when GPSIMD
runs).  Total per-core HBM traffic 64MB (~188us roofline at 358GB/s).
"""

import numpy as np

B, T, H = 4, 4096, 4096
N_CORES = 8
R_TOTAL = B * T            # 16384 rows
FD = R_TOTAL // N_CORES    # 2048 rows per core = free dim
P = 128
NB = H // P                # 32 h-blocks per core
C16 = 1536.0               # f16 rint magic: 1.5 * 2^10
NMM = 512                  # matmul free-dim per PSUM bank

_NC_CACHE: dict = {}


def build_nc():
    """Build the per-core Bass program (identical on all cores)."""
    from contextlib import ExitStack

    import concourse.bacc as bacc
    import concourse.bass as bass
    import concourse.mybir as mybir
    from concourse.tile import TileContext

    f32 = mybir.dt.float32
    f16 = mybir.dt.float16
    bf16 = mybir.dt.bfloat16
    Alu = mybir.AluOpType
    Act = mybir.ActivationFunctionType

    nc = bacc.Bacc("TRN2", target_bir_lowering=False, debug=False)
    xm_d = nc.dram_tensor("xm", [NB * P, 3 * FD], bf16, kind="ExternalInput").ap()
    th_d = nc.dram_tensor("thp", [P, NB], f32, kind="ExternalInput").ap()
    e_d = nc.dram_tensor("eye", [P, P], f16, kind="ExternalInput").ap()
    o_d = nc.dram_tensor("spike", [NB * P, FD], bf16, kind="ExternalOutput").ap()

    with TileContext(nc) as tc, ExitStack() as ctx:
        consts = ctx.enter_context(tc.tile_pool(name="consts", bufs=1))
        io = ctx.enter_context(tc.tile_pool(name="io", bufs=4))
        pm = ctx.enter_context(tc.tile_pool(name="pm", bufs=2))
        pa = ctx.enter_context(tc.tile_pool(name="pa", bufs=2))
        pk = ctx.enter_context(tc.tile_pool(name="pk", bufs=2))
        pj = ctx.enter_context(tc.tile_pool(name="pj", bufs=2))
        ps = ctx.enter_context(tc.tile_pool(name="ps", bufs=2))
        po = ctx.enter_context(tc.tile_pool(name="po", bufs=3))
        psum = ctx.enter_context(tc.tile_pool(name="psum", bufs=2, space="PSUM"))

        # ---- one-time setup ----
        TH = consts.tile([P, NB], f32, tag="TH")
        nc.sync.dma_start(out=TH[:], in_=th_d)
        Rr = consts.tile([P, NB], f32, tag="Rr")
        nc.vector.reciprocal(Rr[:], TH[:])
        Rn = consts.tile([P, NB], f32, tag="Rn")
        nc.vector.tensor_scalar_mul(Rn[:], Rr[:], -1.0)
        BTH = consts.tile([P, NB], f32, tag="BTH")
        nc.vector.tensor_scalar_mul(BTH[:], TH[:], -(2.0 * C16 + 1.0))
        eye = consts.tile([P, P], f16, tag="eye")
        nc.sync.dma_start(out=eye[:], in_=e_d)
        bias_ta = consts.tile([P, 1], f32, tag="bias_ta")
        nc.vector.memset(bias_ta[:], C16 + 0.5)

        xm_t = xm_d.rearrange("(nb p) w -> nb p w", p=P)
        o_t = o_d.rearrange("(nb p) w -> nb p w", p=P)

        for b in range(NB):
            txm = io.tile([P, 3 * FD], bf16, tag="xm")
            nc.sync.dma_start(out=txm[:], in_=xm_t[b])
            xpart = txm[:, 0:FD].bitcast(f16)
            mpart = txm[:, FD : 2 * FD].bitcast(f16)
            scp = txm[:, 2 * FD : 3 * FD]

            # m = x + mem  (DVE TT, f16 2x)
            tm = pm.tile([P, FD], f16, tag="m")
            nc.vector.tensor_tensor(tm[:], xpart, mpart, Alu.add)
            # ta_b = f16(R*m + 1536.5) = 1536 + ceil(q)  (ACT; f16 out = rint)
            ta = pa.tile([P, FD], f16, tag="ta")
            nc.scalar.activation(
                ta[:], tm[:], Act.Identity, bias=bias_ta[:], scale=Rr[:, b : b + 1]
            )
            # kp_b = max(ta_b, 1537) = 1537 + k_pos  (DVE TS 4x)
            kp = pk.tile([P, FD], f16, tag="kp")
            nc.vector.tensor_scalar_max(kp[:], ta[:], C16 + 1.0)
            # jn_b = min(ta_b, 1536) = 1536 + min(g,0)  (DVE TS 4x)
            jn = pj.tile([P, FD], f16, tag="jn")
            nc.vector.tensor_scalar_min(jn[:], ta[:], C16)
            # sn_b = sc*(-R) + 1536 = 1536 - s  (DVE TS 4x)
            sn = ps.tile([P, FD], f16, tag="sn")
            nc.vector.tensor_scalar(
                sn[:], scp, Rn[:, b : b + 1], C16, Alu.mult, Alu.add
            )
            # kn_b = max(jn_b, sn_b) = 1536 - k_neg  (DVE TT, in place)
            nc.vector.tensor_tensor(jn[:], jn[:], sn[:], Alu.max)
            # psum = I.T@kp_b + I.T@kn_b = 3073 + d  (PE; exact small ints)
            td = psum.tile([P, FD], f32, tag="td")
            for c in range(FD // NMM):
                cs = bass.ts(c, NMM)
                nc.tensor.matmul(td[:, cs], eye[:], kp[:, cs], start=True, stop=False)
                nc.tensor.matmul(td[:, cs], eye[:], jn[:, cs], start=False, stop=True)
            # spike = th*psum - 3073*th = th*d  (ACT: PSUM->SBUF, scale+bias)
            tout = po.tile([P, FD], bf16, tag="out")
            nc.scalar.activation(
                tout[:],
                td[:],
                Act.Identity,
                bias=BTH[:, b : b + 1],
                scale=TH[:, b : b + 1],
            )
            nc.scalar.dma_start(out=o_t[b], in_=tout[:])

    return nc


def make_in_maps(x, mem, sc, th):
    """Pack full [R_TOTAL, H] inputs into per-core transposed tensors.

    Per core: xm[b, p, :] = [x_f16 | mem_f16 | sc_bf16] for hidden channel
    h = b*128+p over that core's 2048 rows, so each block is one
    contiguous DMA.
    """
    import ml_dtypes

    x16 = x.astype(np.float16)
    m16 = mem.astype(np.float16)
    s16 = sc.astype(ml_dtypes.bfloat16)
    thp = np.ascontiguousarray(th.reshape(NB, P).T)  # [P, NB] f32
    eye = np.eye(P, dtype=np.float16)

    in_maps = []
    for c in range(N_CORES):
        rs = slice(c * FD, (c + 1) * FD)
        pkd = np.empty((NB, P, 3 * FD), dtype=np.uint16)
        pkd[:, :, 0:FD] = x16[rs].view(np.uint16).reshape(FD, NB, P).transpose(1, 2, 0)
        pkd[:, :, FD : 2 * FD] = (
            m16[rs].view(np.uint16).reshape(FD, NB, P).transpose(1, 2, 0)
        )
        pkd[:, :, 2 * FD : 3 * FD] = (
            s16[rs].view(np.uint16).reshape(FD, NB, P).transpose(1, 2, 0)
        )
        in_maps.append(
            {
                "xm": pkd.reshape(NB * P, 3 * FD).view(ml_dtypes.bfloat16),
                "thp": thp,
                "eye": eye,
            }
        )
    return in_maps


def unpack_out(results):
    """results[c]["spike"] [NB*P, FD] bf16 -> full [B, T, H] f32."""
    outs = []
    for c in range(N_CORES):
        sp = np.asarray(results[c]["spike"]).astype(np.float32)
        # [NB, P, FD] -> [FD, NB, P] -> [FD, H]
        outs.append(sp.reshape(NB, P, FD).transpose(2, 0, 1).reshape(FD, H))
    return np.concatenate(outs, axis=0).reshape(B, T, H)


def kernel(**inputs: np.ndarray) -> np.ndarray:
    from concourse.bass_utils import run_bass_kernel_spmd

    x = np.ascontiguousarray(inputs["x"], dtype=np.float32).reshape(R_TOTAL, H)
    mem = np.ascontiguousarray(inputs["mem"], dtype=np.float32).reshape(R_TOTAL, H)
    sc = np.ascontiguousarray(inputs["spike_count"], dtype=np.float32).reshape(
        R_TOTAL, H
    )
    th = np.ascontiguousarray(inputs["threshold"], dtype=np.float32)

    if "nc" not in _NC_CACHE:
        nc = build_nc()
        nc.finalize()
        _NC_CACHE["nc"] = nc
    nc = _NC_CACHE["nc"]

    in_maps = make_in_maps(x, mem, sc, th)
    res = run_bass_kernel_spmd(nc, in_maps, core_ids=list(range(N_CORES)))
    return unpack_out(res.results)
